# revision 6
# baseline (speedup 1.0000x reference)
"""DOM-masked transformer layer on 8 Trainium2 NeuronCores.

Sharding: 6144 pulses split into 8 shards of 768 queries (sequence
parallel).  pulse_to_dom_idx is sorted, so documents are contiguous
segments (~6 rows each); attention is block-diagonal.  Each query tile of
128 rows only attends inside a 256-wide key window centered on it (valid
while every document segment is <= 65 rows; asserted at runtime).  Masks
are precomputed on the host as additive 0/-1e6 tiles.

Per-core device program (SPMD, identical for all cores):
  qT/kT/v projections (bf16 matmuls) -> banded attention per (tile, head)
  with exp+rowsum fused on ACT, PE-transpose of P, PV in transposed
  layout -> out-proj -> residual+LN1 (f32) -> FFN w/ exact GELU ->
  residual+LN2 -> DMA out.
"""

import sys

if "/opt/trn_rl_repo" not in sys.path:
    sys.path.insert(0, "/opt/trn_rl_repo")

import numpy as np
import ml_dtypes

import concourse.bass as bass
import concourse.mybir as mybir
import concourse.tile as tile
from concourse.bass_utils import run_bass_kernel_spmd
from concourse.masks import make_identity

# problem constants (hardcoded per contract)
N = 6144
D = 256
H = 4
DH = 64
DFF = 1024
NCORES = 8
NQ = N // NCORES            # 768 queries per core
PAD = 64                    # halo on each side of a query tile
HALO = NQ + 2 * PAD         # 896 rows of K/V context per core
NT = NQ // 128              # 6 query tiles of 128 per core
KW = 256                    # key window per query tile
LN_EPS = 1e-5
MASK_NEG = -1.0e6           # additive mask before exp (scaled by 0.125)

F32 = mybir.dt.float32
BF16 = mybir.dt.bfloat16
AF = mybir.ActivationFunctionType

_CACHE = {}


def _split_excess_waits(nc, max_waits=1):
    """Walrus on this toolchain rejects >1 semaphore wait on one
    instruction; move the excess onto nop carriers inserted just before."""
    for fn in nc.m.functions:
        for bb in fn.blocks:
            insts = list(bb.instructions)
            need = [
                i for i in insts
                if i.sync_info and i.sync_info.on_wait
                and len(i.sync_info.on_wait) > max_waits
            ]
            if not need:
                continue
            new_list = []
            carriers = {}
            for inst in insts:
                if inst.sync_info and inst.sync_info.on_wait and len(
                    inst.sync_info.on_wait
                ) > max_waits:
                    waits = list(inst.sync_info.on_wait)
                    extra = waits[max_waits:]
                    inst.sync_info.on_wait = waits[:max_waits]
                    eng = nc.engines[inst.engine]
                    for j in range(0, len(extra), max_waits):
                        nop = eng.nop()
                        chunk = extra[j : j + max_waits]
                        import bass_rust

                        nop.ins.sync_info = bass_rust.SyncInfo(
                            on_wait=chunk, on_update=[]
                        )
                        carriers.setdefault(inst.name, []).append(nop.ins)
                    new_list.extend(carriers[inst.name])
                new_list.append(inst)
            # the nops were appended to the current bb by emission; drop them
            # from wherever they landed and keep only our ordered copy
            all_carriers = {c.name for cs in carriers.values() for c in cs}
            for fn2 in nc.m.functions:
                for bb2 in fn2.blocks:
                    if bb2 is not bb:
                        bb2.instructions = [
                            i for i in bb2.instructions
                            if i.name not in all_carriers
                        ]
            tail = [i for i in bb.instructions if i.name in all_carriers]
            keep = set(i.name for i in new_list)
            rest = [
                i for i in bb.instructions
                if i.name not in keep and i.name not in all_carriers
            ]
            assert not rest, "unexpected new instructions during split"
            del tail
            bb.instructions = new_list


def _build_bass():
    nc = bass.Bass(target_bir_lowering=False)

    xT = nc.dram_tensor("xT", [D, HALO], BF16, kind="ExternalInput")
    x_own = nc.dram_tensor("x_own", [NQ, D], F32, kind="ExternalInput")
    qkvw = nc.dram_tensor("qkvw", [D, 3 * D], BF16, kind="ExternalInput")
    outw = nc.dram_tensor("outw", [D, D], BF16, kind="ExternalInput")
    w1 = nc.dram_tensor("w1", [D, DFF], BF16, kind="ExternalInput")
    w2 = nc.dram_tensor("w2", [DFF, D], BF16, kind="ExternalInput")
    maskb = nc.dram_tensor("maskb", [NT, 128, KW], F32, kind="ExternalInput")
    out = nc.dram_tensor("out", [NQ, D], F32, kind="ExternalOutput")

    with tile.TileContext(nc) as tc:
        with (
            tc.tile_pool(name="singles", bufs=1) as singles,
            tc.tile_pool(name="wtiles", bufs=2) as wtiles,
            tc.tile_pool(name="attn", bufs=3) as attn,
            tc.tile_pool(name="small", bufs=4) as small,
            tc.tile_pool(name="ps_big", bufs=3, space="PSUM") as ps_big,
            tc.tile_pool(name="ps_sml", bufs=5, space="PSUM") as ps_sml,
        ):
            # ---- constants / weights ----
            ident = singles.tile([128, 128], BF16)
            make_identity(nc, ident)
            eps_sb = singles.tile([128, 1], F32)
            nc.vector.memset(eps_sb, LN_EPS)

            qkvw_sb = []
            for c in range(2):
                t = singles.tile([128, 3 * D], BF16, tag=f"qkvw{c}")
                nc.sync.dma_start(out=t, in_=qkvw[c * 128 : (c + 1) * 128, :])
                qkvw_sb.append(t)
            outw_sb = []
            for c in range(2):
                t = singles.tile([128, D], BF16, tag=f"outw{c}")
                nc.sync.dma_start(out=t, in_=outw[c * 128 : (c + 1) * 128, :])
                outw_sb.append(t)
            w1_sb = []
            for c in range(2):
                t = singles.tile([128, DFF], BF16, tag=f"w1_{c}")
                nc.sync.dma_start(out=t, in_=w1[c * 128 : (c + 1) * 128, :])
                w1_sb.append(t)
            w2_sb = singles.tile([128, 8, D], BF16)
            nc.sync.dma_start(
                out=w2_sb, in_=w2.rearrange("(c p) n -> p c n", p=128)
            )
            xT_sb = []
            for c in range(2):
                t = singles.tile([128, HALO], BF16, tag=f"xT{c}")
                nc.sync.dma_start(out=t, in_=xT[c * 128 : (c + 1) * 128, :])
                xT_sb.append(t)

            # ---- projections: qT, kT (T layout), v (normal layout) ----
            # qT[dq, n] = sum_d Wq[d, dq] * xT[d, n]
            qT_sb = [singles.tile([128, HALO], BF16, tag=f"qT{c}", name=f"qT{c}") for c in range(2)]
            kT_sb = [singles.tile([128, HALO], BF16, tag=f"kT{c}", name=f"kT{c}") for c in range(2)]
            NCH = 448  # free-dim chunk for projection matmuls
            for oc in range(4):  # dq 0:128,128:256, dk 0:128,128:256
                dst = qT_sb[oc] if oc < 2 else kT_sb[oc - 2]
                wcols = slice(oc * 128, (oc + 1) * 128)
                for nchunk in range(2):
                    ncols = slice(nchunk * NCH, (nchunk + 1) * NCH)
                    ps = ps_big.tile([128, NCH], F32, tag="psA")
                    nc.tensor.matmul(
                        ps, qkvw_sb[0][:, wcols], xT_sb[0][:, ncols],
                        start=True, stop=False,
                    )
                    nc.tensor.matmul(
                        ps, qkvw_sb[1][:, wcols], xT_sb[1][:, ncols],
                        start=False, stop=True,
                    )
                    nc.any.tensor_copy(out=dst[:, ncols], in_=ps)

            # v[n, dv] = sum_d xT[d, n]^T Wv[d, dv], normal layout chunks
            v_sb = singles.tile([128, 7, D], BF16)
            for c in range(7):
                ncols = slice(c * 128, (c + 1) * 128)
                ps = ps_big.tile([128, D], F32, tag="psA")
                nc.tensor.matmul(
                    ps, xT_sb[0][:, ncols], qkvw_sb[0][:, 2 * D : 3 * D],
                    start=True, stop=False,
                )
                nc.tensor.matmul(
                    ps, xT_sb[1][:, ncols], qkvw_sb[1][:, 2 * D : 3 * D],
                    start=False, stop=True,
                )
                nc.any.tensor_copy(out=v_sb[:, c, :], in_=ps)

            # ---- attention + rest, per query tile ----
            x2_all = singles.tile([128, NT, D], F32)   # LN1 output, f32
            x2T_sb = [singles.tile([128, NQ], BF16, tag=f"x2T{c}", name=f"x2T{c}") for c in range(2)]
            hT_sb = singles.tile([128, 8, NQ], BF16)   # gelu(ffn1), T layout

            for t in range(NT):
                qcols = slice(PAD + 128 * t, PAD + 128 * t + 128)
                kwin = slice(128 * t, 128 * t + KW)

                mb = attn.tile([128, KW], F32, tag="maskb")
                nc.sync.dma_start(out=mb, in_=maskb[t, :, :])

                aoT = [attn.tile([128, 128], BF16, tag=f"aoT{c}", name=f"aoT{c}") for c in range(2)]
                for h in range(H):
                    qrows = slice((h % 2) * DH, (h % 2) * DH + DH)
                    qt = qT_sb[h // 2]
                    kt = kT_sb[h // 2]
                    ps_s = ps_big.tile([128, KW], F32, tag="psA")
                    nc.tensor.matmul(
                        ps_s, qt[qrows, qcols], kt[qrows, kwin],
                        start=True, stop=True,
                    )
                    nc.vector.tensor_add(ps_s, ps_s, mb)
                    e_sb = attn.tile([128, KW], BF16, tag="e")
                    sums = small.tile([128, 1], F32, tag="sums")
                    nc.scalar.activation(
                        out=e_sb, in_=ps_s, func=AF.Exp, scale=0.125,
                        accum_out=sums,
                    )
                    recip = small.tile([128, 1], F32, tag="recip")
                    nc.vector.reciprocal(recip, sums)
                    nc.vector.tensor_scalar_mul(e_sb, e_sb, recip)
                    # transpose P: [128q, 256k] -> 2x [128k, 128q]
                    pt_sb = []
                    for c in range(2):
                        ps_t = ps_sml.tile([128, 128], BF16, tag="psB")
                        nc.tensor.transpose(
                            ps_t, e_sb[:, c * 128 : (c + 1) * 128], ident
                        )
                        pt = attn.tile([128, 128], BF16, tag=f"pt{c}")
                        nc.any.tensor_copy(out=pt, in_=ps_t)
                        pt_sb.append(pt)
                    # attn_out^T[dh, q] = sum_k v[k, dh] * P^T[k, q]
                    ps_o = ps_sml.tile([64, 128], F32, tag="psB")
                    hc = slice(h * DH, (h + 1) * DH)
                    nc.tensor.matmul(
                        ps_o, v_sb[:, t, hc], pt_sb[0], start=True, stop=False
                    )
                    nc.tensor.matmul(
                        ps_o, v_sb[:, t + 1, hc], pt_sb[1], start=False, stop=True
                    )
                    dst = aoT[h // 2]
                    drows = slice((h % 2) * DH, (h % 2) * DH + DH)
                    nc.any.tensor_copy(out=dst[drows, :], in_=ps_o)

                # out-projection: y[q, d] = sum_din aoT[din, q] * outw[din, d]
                ps_y = ps_big.tile([128, D], F32, tag="psA")
                nc.tensor.matmul(ps_y, aoT[0], outw_sb[0], start=True, stop=False)
                nc.tensor.matmul(ps_y, aoT[1], outw_sb[1], start=False, stop=True)

                # residual + LN1 (f32)
                x_sb = attn.tile([128, D], F32, tag="x_sb")
                nc.sync.dma_start(out=x_sb, in_=x_own[t * 128 : (t + 1) * 128, :])
                x1 = attn.tile([128, D], F32, tag="x1")
                nc.vector.tensor_add(x1, ps_y, x_sb)

                stats = small.tile([128, 6], F32, tag="stats")
                nc.vector.bn_stats(out=stats, in_=x1)
                mv = small.tile([128, 2], F32, tag="mv")
                nc.vector.bn_aggr(out=mv, in_=stats)
                rstd = small.tile([128, 1], F32, tag="rstd")
                nc.scalar.activation(
                    out=rstd, in_=mv[:, 1:2], func=AF.Sqrt, bias=eps_sb
                )
                nc.vector.reciprocal(rstd, rstd)
                nmb = small.tile([128, 1], F32, tag="nmb")
                nc.vector.tensor_mul(nmb, mv[:, 0:1], rstd)
                nc.vector.tensor_scalar_mul(nmb, nmb, -1.0)
                x2 = x2_all[:, t, :]
                nc.scalar.activation(
                    out=x2, in_=x1, func=AF.Identity, bias=nmb, scale=rstd
                )
                # x2 -> bf16 -> transpose into x2T
                x2b = attn.tile([128, D], BF16, tag="x2b")
                nc.any.tensor_copy(out=x2b, in_=x2)
                for c in range(2):
                    ps_t2 = ps_sml.tile([128, 128], BF16, tag="psB")
                    nc.tensor.transpose(
                        ps_t2, x2b[:, c * 128 : (c + 1) * 128], ident
                    )
                    nc.any.tensor_copy(
                        out=x2T_sb[c][:, t * 128 : (t + 1) * 128], in_=ps_t2
                    )

            # ---- FFN1 + gelu: hT[dff, q] ----
            FCH = 384
            for f in range(8):
                fcols = slice(f * 128, (f + 1) * 128)
                for nchunk in range(2):
                    ncols = slice(nchunk * FCH, (nchunk + 1) * FCH)
                    ps_h = ps_big.tile([128, FCH], F32, tag="psA")
                    nc.tensor.matmul(
                        ps_h, w1_sb[0][:, fcols], x2T_sb[0][:, ncols],
                        start=True, stop=False,
                    )
                    nc.tensor.matmul(
                        ps_h, w1_sb[1][:, fcols], x2T_sb[1][:, ncols],
                        start=False, stop=True,
                    )
                    nc.scalar.activation(
                        out=hT_sb[:, f, ncols], in_=ps_h, func=AF.Gelu
                    )

            # ---- FFN2 + residual + LN2 + store ----
            for t in range(NT):
                ps_y2 = ps_big.tile([128, D], F32, tag="psA")
                for f in range(8):
                    nc.tensor.matmul(
                        ps_y2,
                        hT_sb[:, f, t * 128 : (t + 1) * 128],
                        w2_sb[:, f, :],
                        start=(f == 0), stop=(f == 7),
                    )
                x3 = attn.tile([128, D], F32, tag="x3")
                nc.vector.tensor_add(x3, ps_y2, x2_all[:, t, :])

                stats2 = small.tile([128, 6], F32, tag="stats2")
                nc.vector.bn_stats(out=stats2, in_=x3)
                mv2 = small.tile([128, 2], F32, tag="mv2")
                nc.vector.bn_aggr(out=mv2, in_=stats2)
                rstd2 = small.tile([128, 1], F32, tag="rstd2")
                nc.scalar.activation(
                    out=rstd2, in_=mv2[:, 1:2], func=AF.Sqrt, bias=eps_sb
                )
                nc.vector.reciprocal(rstd2, rstd2)
                nmb2 = small.tile([128, 1], F32, tag="nmb2")
                nc.vector.tensor_mul(nmb2, mv2[:, 0:1], rstd2)
                nc.vector.tensor_scalar_mul(nmb2, nmb2, -1.0)
                o_sb = attn.tile([128, D], F32, tag="o_sb")
                nc.scalar.activation(
                    out=o_sb, in_=x3, func=AF.Identity, bias=nmb2, scale=rstd2
                )
                nc.sync.dma_start(out=out[t * 128 : (t + 1) * 128, :], in_=o_sb)

    _split_excess_waits(nc)
    return nc


def _host_prep(x, pulse_to_dom_idx, qkv_w, out_w, ff_w1, ff_w2):
    bf = ml_dtypes.bfloat16
    dom = np.asarray(pulse_to_dom_idx)
    # document segments must fit the 64-row halo
    _, counts = np.unique(dom, return_counts=True)
    assert counts.max() <= PAD + 1, f"doc segment too long: {counts.max()}"

    xpad = np.zeros((N + 2 * PAD, D), np.float32)
    xpad[PAD : PAD + N] = x
    dompad = np.full(N + 2 * PAD, -1, np.int64)
    dompad[PAD : PAD + N] = dom

    in_maps = []
    for c in range(NCORES):
        h0 = c * NQ  # padded-row index of halo start
        xT_c = np.ascontiguousarray(xpad[h0 : h0 + HALO].T.astype(bf))
        x_own_c = np.ascontiguousarray(xpad[h0 + PAD : h0 + PAD + NQ])
        mb = np.empty((NT, 128, KW), np.float32)
        for t in range(NT):
            qs = h0 + PAD + 128 * t          # padded idx of first query row
            ks = h0 + 128 * t                # padded idx of window start
            same = dompad[qs : qs + 128, None] == dompad[None, ks : ks + KW]
            mb[t] = np.where(same, 0.0, MASK_NEG)
        in_maps.append(
            {
                "xT": xT_c,
                "x_own": x_own_c,
                "qkvw": np.ascontiguousarray(qkv_w.astype(bf)),
                "outw": np.ascontiguousarray(out_w.astype(bf)),
                "w1": np.ascontiguousarray(ff_w1.astype(bf)),
                "w2": np.ascontiguousarray(ff_w2.astype(bf)),
                "maskb": mb,
            }
        )
    return in_maps


def _make_runner(nc, n_cores=NCORES):
    """Compile the bass module via the bass2jax/PJRT path once and return
    (prep, execute, collect): prep(in_maps) -> flat input list,
    execute(flat) -> jax out arrays, collect(outs) -> full output."""
    import jax
    from jax.sharding import Mesh, PartitionSpec
    from jax.experimental.shard_map import shard_map
    from concourse import bass2jax as b2j

    b2j.install_neuronx_cc_hook()

    partition_name = nc.partition_id_tensor.name if nc.partition_id_tensor else None
    in_names, out_names, out_avals, zero_outs = [], [], [], []
    for alloc in nc.m.functions[0].allocations:
        if not isinstance(alloc, mybir.MemoryLocationSet):
            continue
        name = alloc.memorylocations[0].name
        if alloc.kind == "ExternalInput":
            if name != partition_name:
                in_names.append(name)
        elif alloc.kind == "ExternalOutput":
            out_names.append(name)
            shape = tuple(alloc.tensor_shape)
            dtype = mybir.dt.np(alloc.dtype)
            out_avals.append(jax.core.ShapedArray(shape, dtype))
            zero_outs.append(np.zeros(shape, dtype))
    n_params = len(in_names)
    n_outs = len(out_avals)
    all_in_names = list(in_names) + list(out_names)
    if partition_name is not None:
        all_in_names.append(partition_name)
    donate = tuple(range(n_params, n_params + n_outs))

    def _body(*args):
        operands = list(args)
        if partition_name is not None:
            operands.append(b2j.partition_id_tensor())
        outs = b2j._bass_exec_p.bind(
            *operands,
            out_avals=tuple(out_avals),
            in_names=tuple(all_in_names),
            out_names=tuple(out_names),
            lowering_input_output_aliases=(),
            sim_require_finite=True,
            sim_require_nnan=True,
            nc=nc,
        )
        return tuple(outs)

    devices = jax.devices()[:n_cores]
    mesh = Mesh(np.asarray(devices), ("core",))
    in_specs = (PartitionSpec("core"),) * (n_params + n_outs)
    out_specs = (PartitionSpec("core"),) * n_outs
    sharded = jax.jit(
        shard_map(
            _body, mesh=mesh, in_specs=in_specs, out_specs=out_specs,
            check_rep=False,
        ),
        donate_argnums=donate, keep_unused=True,
    )

    def prep(in_maps):
        per_core = [[np.asarray(m[name]) for name in in_names] for m in in_maps]
        flat = [
            np.concatenate([per_core[c][i] for c in range(n_cores)], axis=0)
            for i in range(n_params)
        ]
        return flat

    def execute(flat):
        concat_zeros = [
            np.zeros((n_cores * z.shape[0], *z.shape[1:]), z.dtype)
            for z in zero_outs
        ]
        outs = sharded(*flat, *concat_zeros)
        jax.block_until_ready(outs)
        return outs

    def collect(outs):
        # single output "out": [n_cores*NQ, D] -> full
        return np.asarray(outs[0])

    return prep, execute, collect


def _get_runner():
    if "runner" not in _CACHE:
        _CACHE["nc"] = _build_bass()
        _CACHE["runner"] = _make_runner(_CACHE["nc"])
    return _CACHE["runner"]


def kernel(
    x, pulse_to_dom_idx, qkv_w, qkv_b, out_w, out_b,
    ff_w1, ff_b1, ff_w2, ff_b2, ln1_g, ln1_b, ln2_g, ln2_b,
):
    x = np.asarray(x, np.float32)
    for b in (qkv_b, out_b, ff_b1, ff_b2, ln1_b, ln2_b):
        assert np.abs(np.asarray(b)).max() == 0.0, "nonzero bias unsupported"
    for g in (ln1_g, ln2_g):
        assert np.abs(np.asarray(g) - 1.0).max() == 0.0, "ln gain unsupported"

    prep, execute, collect = _get_runner()
    in_maps = _host_prep(
        x, pulse_to_dom_idx,
        np.asarray(qkv_w, np.float32), np.asarray(out_w, np.float32),
        np.asarray(ff_w1, np.float32), np.asarray(ff_w2, np.float32),
    )
    return collect(execute(prep(in_maps)))
